# revision 39
# baseline (speedup 1.0000x reference)
"""ConvFormer Trainium2 kernel — data-parallel over B across 8 NeuronCores.

Reference (per batch element b):
    x1 = x[b].T                                   # (D, L) channel-major
    for (W, bias) in ((W3,b3),(W5,b5),(W7,b7)):   # chained masked convs
        x1 = bias + sum_k mask_k * (W[:,:,k] @ shift(x1, k))
    h  = LN(x[b] + x1.T)                          # (L, D)
    out = LN(h + gelu(h @ w1 + b1) @ w2 + b2)

v2 design (per core), all bf16 matmuls (PE streams bf16 at the same
1 col/cycle as f32r, but halves SBUF/DMA/DVE cost). Measured ~681us
(vs 1052us f32r baseline); all matmul phases stream at the 216ns/MM
N=512 hardware floor, 96.5% of bf16 peak FLOPs during the busy window.
  - conv stages ping-pong between two SBUF-resident bf16 tensors
    [128, CD, PL] — no HBM bounce, no inter-conv DMA latency on the
    dependency chain, PE stays warm (HAM never re-throttles).
  - masked conv via masked-input trick: Z_d = shift(x,d) * m_d computed on
    DVE in bf16; all (tap, c) matmuls accumulate in PSUM. One shared 8-bank
    PSUM pool rotates conv blocks (4 banks each, ping-pong) and the MLP
    tiles with zero bank-reuse stalls.
  - convs 1,2 channel-major -> channel-major (copyout fuses the conv bias and
    writes straight into the next stage tensor); conv 3 channel-major ->
    L-major, landing in LayerNorm layout; residual (x + b7, host-folded,
    f32 from HBM) fused into the copyout.
  - LN via bn_stats/bn_aggr; per block/lg the 4 chunks' sqrt+reciprocal are
    batched into one strided op each; h stored bf16, bounced via HBM.
  - MLP: PE-transpose h -> hc; MLP1 out = w1.T @ hc (hidden-major), gelu
    fused on ScalarE; MLP2 uses hids as the *stationary* operand and w2 as
    moving, producing the output directly in L-major — no transpose-back.
  - LN2 finalize (sqrt/recip/normalize/store) deferred one lg so the scalar
    FIFO never blocks the next lg's gelus behind a Sqrt.
  - single flat pool scope: phase 2 instructions interleave with the conv3
    tail; all weights prefetched during phase 1. Startup DMAs split into
    small pieces across the two HWDGE queues (SP=masks, Act=weights+x3) and
    the gpsimd SWDGE queue (x0-2) — first matmul at ~8us after the engine
    preamble, which is the 2-fast-queue bandwidth bound.
  - ScalarE does only Sqrt in phase 1 and Gelu+Sqrt in phase 2 (act-table
    thrash minimized); all PSUM copies/adds on DVE (gpsimd has no PSUM port,
    and its tensor ops are ~5x slower than DVE — keep it to DMA issue only).
"""

import numpy as np

B, L, D = 8, 4096, 512
KS = (3, 5, 7)
EPS = 1e-5
NCORES = 8
PAD = 8            # zero-pad columns each side (PL % 16 == 0, fp8-ready)
PL = PAD + L + PAD
BLK = 512          # conv L-block size
NBLK = L // BLK
CD = D // 128      # 4 channel chunks
LCH = L // 128     # 32 L-chunks of 128
H = 4 * D          # mlp hidden
JD = H // 128      # 16 hidden chunks
GELU_FUNC_NAME = "Gelu_apprx_tanh"  # jax.nn.gelu default is approximate=True

_CACHE = {}


def _build_nc(ln1_affine, ln2_affine, b2_nonzero):
    import concourse.bass as bass
    import concourse.tile as tile
    from concourse import bacc, mybir
    from concourse.masks import make_identity

    f32 = mybir.dt.float32
    bf16 = mybir.dt.bfloat16
    GELU = getattr(mybir.ActivationFunctionType, GELU_FUNC_NAME)
    SQRT = mybir.ActivationFunctionType.Sqrt
    ADD = mybir.AluOpType.add
    SUB = mybir.AluOpType.subtract
    MULT = mybir.AluOpType.mult

    # global tap -> (conv index, shift d); convs use taps [0:3], [3:8], [8:15]
    conv_taps = []
    t0 = 0
    for ki, K in enumerate(KS):
        p = (K - 1) // 2
        conv_taps.append([(t0 + i, i - p) for i in range(K)])
        t0 += K
    NT = t0  # 15
    tap_off = [0, 3, 8]
    # mask row index for shift d (d != 0)
    d2m = {-3: 0, -2: 1, -1: 2, 1: 3, 2: 4, 3: 5}

    nc = bacc.Bacc(None, target_bir_lowering=False)

    xcp = nc.declare_dram_parameter("xcp", [CD, 128, PL], bf16, isOutput=False)
    xb = nc.declare_dram_parameter("xb", [LCH, 128, D], f32, isOutput=False)
    masks = nc.declare_dram_parameter("masks", [6, L], bf16, isOutput=False)
    wc = nc.declare_dram_parameter("wc", [128, NT * CD * D], bf16, isOutput=False)
    cb1 = nc.declare_dram_parameter("cb1", [128, CD], f32, isOutput=False)
    cb2 = nc.declare_dram_parameter("cb2", [128, CD], f32, isOutput=False)
    w1 = nc.declare_dram_parameter("w1", [128, CD * H], bf16, isOutput=False)
    b1c = nc.declare_dram_parameter("b1c", [128, JD], f32, isOutput=False)
    w2 = nc.declare_dram_parameter("w2", [128, JD * D], bf16, isOutput=False)
    b2m = nc.declare_dram_parameter("b2m", [1, D], f32, isOutput=False)
    if ln1_affine:
        g1r = nc.declare_dram_parameter("g1r", [1, D], f32, isOutput=False)
        b1r = nc.declare_dram_parameter("b1r", [1, D], f32, isOutput=False)
    if ln2_affine:
        g2r = nc.declare_dram_parameter("g2r", [1, D], f32, isOutput=False)
        b2r = nc.declare_dram_parameter("b2r", [1, D], f32, isOutput=False)
    out = nc.declare_dram_parameter("out", [L, D], f32, isOutput=True)

    hbuf = nc.dram_tensor("hbuf", [LCH, 128, D], bf16)

    def bcast_row_ap(param, row, col0, n):
        """DMA access pattern: one DRAM row slice broadcast to 128 partitions."""
        src = param[row, col0:col0 + n]
        return bass.AP(tensor=src.tensor, offset=src.offset, ap=[[0, 128]] + list(src.ap))

    from contextlib import ExitStack

    with tile.TileContext(nc) as tc, ExitStack() as es:
        def mkpool(name, bufs, space="SBUF"):
            return es.enter_context(tc.tile_pool(name=name, bufs=bufs, space=space))

        const = mkpool("const", 1)
        stpool = mkpool("stage", 2)
        wpool = mkpool("wconv", 2)
        wmpool = mkpool("wmlp", 1)
        mpool = mkpool("maskp", 4)
        zpool = mkpool("zp", 10)
        xbpool = mkpool("xbp", 2)
        spool = mkpool("sp", 5)
        statpool = mkpool("stats", 8)
        hpool = mkpool("hp", 3)
        hinpool = mkpool("hin", 5)
        hcpool = mkpool("hcp", 1)
        hidpool = mkpool("hid", 1)
        s2pool = mkpool("s2p", 8)
        outpool = mkpool("outp", 3)
        psp = mkpool("psp", 8, space="PSUM")

        if True:
            # ---- input / weight prefetch (many queues, earliest first) ----
            stages = [
                stpool.tile([128, CD, PL], bf16, tag="stage", name=f"stage{i}")
                for i in range(2)
            ]
            sA, sB = stages
            # x lands in stage A as quarter-DMAs, earliest columns first:
            # chunks 0-2 on the gpsimd queue, chunk 3 interleaved with the
            # conv1 weight taps on the scalar queue. Only gpsimd/SP/Act can
            # issue DMAs; sync stays free for the (tiny, critical) masks.
            QL = PL // 4
            def xq(c, q):
                return dict(out=sA[:, c, q * QL:(q + 1) * QL],
                            in_=xcp[c, :, q * QL:(q + 1) * QL])
            # conv1's 3 weight taps load as separate DMAs so the first
            # matmul only waits ~0.5MB; conv3 (7 taps) is split into
            # 4+3-tap tiles so the wconv slot is only 5 taps wide.
            wts = [[], [], []]
            w0 = wpool.tile([128, 3 * CD * D], bf16, tag="wconv", name="wc0")
            wts[0].append((w0, 3))
            TW = CD * D
            nc.scalar.dma_start(out=w0[:, 0:TW], in_=wc[:, 0:TW])
            EL = QL // 2
            for c in range(3):
                nc.gpsimd.dma_start(out=sA[:, c, 0:EL], in_=xcp[c, :, 0:EL])
            for c in range(3):
                nc.gpsimd.dma_start(out=sA[:, c, EL:QL], in_=xcp[c, :, EL:QL])
            for q in range(4):
                if q > 0:
                    for c in range(3):
                        nc.gpsimd.dma_start(**xq(c, q))
                nc.scalar.dma_start(**xq(3, q))
                if q < 2:
                    nc.scalar.dma_start(out=w0[:, (q + 1) * TW:(q + 2) * TW],
                                        in_=wc[:, (q + 1) * TW:(q + 2) * TW])
            for ci, parts, a0 in ((1, (5,), 3 * TW), (2, (4, 3), 8 * TW)):
                for pi, ntap in enumerate(parts):
                    wt = wpool.tile([128, ntap * CD * D], bf16, tag="wconv",
                                    name=f"wc{ci}_{pi}")
                    nc.scalar.dma_start(out=wt[:], in_=wc[:, a0:a0 + ntap * CD * D])
                    a0 += ntap * CD * D
                    wts[ci].append((wt, ntap))
            w1t = wmpool.tile([128, CD, H], bf16, tag="w1")
            for q in range(2):
                nc.scalar.dma_start(
                    out=w1t[:, 2 * q:2 * q + 2, :],
                    in_=w1[:, q * 2 * H:(q + 1) * 2 * H],
                )
            w2t = wmpool.tile([128, JD, D], bf16, tag="w2")
            for q in range(2):
                nc.scalar.dma_start(
                    out=w2t[:, 8 * q:8 * q + 8, :],
                    in_=w2[:, q * 8 * D:(q + 1) * 8 * D],
                )

            # ---- constants ----
            ident = const.tile([128, 128], bf16)
            make_identity(nc, ident)
            epst = const.tile([128, 1], f32)
            nc.vector.memset(epst, EPS)
            cb1t = const.tile([128, CD], f32)
            nc.sync.dma_start(out=cb1t[:], in_=cb1[:])
            cb2t = const.tile([128, CD], f32)
            nc.sync.dma_start(out=cb2t[:], in_=cb2[:])
            b1ct = const.tile([128, JD], f32)
            nc.sync.dma_start(out=b1ct[:], in_=b1c[:])
            if b2_nonzero:
                b2bt = const.tile([128, D], f32)
                nc.sync.dma_start(out=b2bt[:], in_=bcast_row_ap(b2m, 0, 0, D))
            if ln1_affine:
                g1t = const.tile([128, D], f32)
                nc.sync.dma_start(out=g1t[:], in_=bcast_row_ap(g1r, 0, 0, D))
                b1t = const.tile([128, D], f32)
                nc.sync.dma_start(out=b1t[:], in_=bcast_row_ap(b1r, 0, 0, D))
            if ln2_affine:
                g2t = const.tile([128, D], f32)
                nc.sync.dma_start(out=g2t[:], in_=bcast_row_ap(g2r, 0, 0, D))
                b2t = const.tile([128, D], f32)
                nc.sync.dma_start(out=b2t[:], in_=bcast_row_ap(b2r, 0, 0, D))
            # zero the pad columns of both stage tensors once
            for s in stages:
                for c in range(CD):
                    nc.vector.memset(s[:, c, 0:PAD], 0.0)
                    nc.vector.memset(s[:, c, PAD + L:PL], 0.0)

            # ---------------- Phase 1: conv chain + LN1 -> hbuf ----------------
            for ci, (K, taps) in enumerate(zip(KS, conv_taps)):
                src = stages[ci % 2]
                dst = stages[(ci + 1) % 2]  # unused for ci == 2
                tapw = []  # tap -> (weight tile, local tap index)
                for wt, ntap in wts[ci]:
                    for i in range(ntap):
                        tapw.append((wt, i))
                for blk in range(NBLK):
                    l0 = blk * BLK
                    # masked shifted inputs for this block (center tap d=0
                    # reads the stage directly)
                    zcs = {}
                    for ti, (t, d) in enumerate(taps):
                        if d == 0:
                            zcs[ti] = [
                                src[:, c, PAD + l0:PAD + l0 + BLK] for c in range(CD)
                            ]
                        else:
                            mt = mpool.tile([128, BLK], bf16, tag="maskp")
                            nc.sync.dma_start(
                                out=mt[:], in_=bcast_row_ap(masks, d2m[d], l0, BLK)
                            )
                            zs = []
                            for c in range(CD):
                                zt = zpool.tile([128, BLK], bf16, tag="zp")
                                nc.vector.tensor_tensor(
                                    out=zt[:],
                                    in0=src[:, c, PAD + l0 + d:PAD + l0 + d + BLK],
                                    in1=mt[:],
                                    op=MULT,
                                )
                                zs.append(zt)
                            zcs[ti] = zs
                    pss = [
                        psp.tile([128, BLK], f32, tag="ps", name=f"ps{i}")
                        for i in range(4)
                    ]
                    for ti, (t, d) in enumerate(taps):
                        wt, lt = tapw[ti]
                        for c in range(CD):
                            kw = dict(
                                start=(ti == 0 and c == 0),
                                stop=(ti == K - 1 and c == CD - 1),
                                skip_group_check=True,
                            )
                            wbase = (lt * CD + c) * D
                            if ci < 2:
                                # CM: out[o,l]; lhsT = W[c,o]; rhs = Z[c,l]
                                for o in range(CD):
                                    nc.tensor.matmul(
                                        pss[o][:],
                                        wt[:, wbase + o * 128:wbase + (o + 1) * 128],
                                        zcs[ti][c],
                                        **kw,
                                    )
                            else:
                                # LM: out[l,o]; lhsT = Z[c,l-slice]; rhs = W[c,:]
                                for lch in range(BLK // 128):
                                    nc.tensor.matmul(
                                        pss[lch][:],
                                        zcs[ti][c][:, lch * 128:(lch + 1) * 128],
                                        wt[:, wbase:wbase + D],
                                        **kw,
                                    )

                    if ci < 2:
                        bct = cb1t if ci == 0 else cb2t
                        for o in range(CD):
                            nc.vector.tensor_scalar(
                                out=dst[:, o, PAD + l0:PAD + l0 + BLK],
                                in0=pss[o][:],
                                scalar1=bct[:, o:o + 1], scalar2=None, op0=ADD,
                            )
                    else:
                        # LN1 for the 4 chunks of this block: stats first
                        # (DVE), then ONE batched sqrt + reciprocal, then the
                        # 4 normalizes — avoids serializing 4 cross-engine
                        # sqrt round-trips.
                        sts = []
                        mvs = statpool.tile([128, 8], f32, tag="mv")
                        for lch in range(BLK // 128):
                            lg = (l0 + lch * 128) // 128
                            ps = pss[lch]
                            xbt = xbpool.tile([128, D], f32, tag="xbp")
                            nc.gpsimd.dma_start(out=xbt[:], in_=xb[lg])
                            st = spool.tile([128, D], bf16, tag="sp")
                            nc.vector.scalar_tensor_tensor(
                                out=st[:], in0=ps[:], scalar=1.0, in1=xbt[:],
                                op0=MULT, op1=ADD,
                            )
                            sts.append(st)
                            stats = statpool.tile([128, 6], f32, tag="st6")
                            nc.vector.bn_stats(out=stats[:], in_=st[:])
                            nc.vector.bn_aggr(out=mvs[:, 2 * lch:2 * lch + 2], in_=stats[:])
                        stds = statpool.tile([128, 4], f32, tag="sd")
                        nc.scalar.activation(
                            out=stds[:], in_=mvs[:, 1::2], func=SQRT,
                            bias=epst[:], scale=1.0,
                        )
                        nc.vector.reciprocal(out=stds[:], in_=stds[:])
                        for lch in range(BLK // 128):
                            lg = (l0 + lch * 128) // 128
                            ht = hpool.tile([128, D], bf16, tag="hp")
                            nc.vector.tensor_scalar(
                                out=ht[:], in0=sts[lch],
                                scalar1=mvs[:, 2 * lch:2 * lch + 1],
                                scalar2=stds[:, lch:lch + 1],
                                op0=SUB, op1=MULT,
                            )
                            if ln1_affine:
                                nc.vector.tensor_tensor(out=ht[:], in0=ht[:], in1=g1t[:], op=MULT)
                                nc.vector.tensor_tensor(out=ht[:], in0=ht[:], in1=b1t[:], op=ADD)
                            nc.gpsimd.dma_start(out=hbuf[lg], in_=ht[:])

            # ---------------- Phase 2: MLP + LN2 -> out ----------------
            # LN2's finalize (sqrt/recip/normalize/store) for lg is deferred
            # until after lg+1's gelu burst: keeps the scalar FIFO from
            # blocking lg+1's gelus (and hence its MLP2) behind lg's Sqrt.
            out_eng = [nc.scalar, nc.sync]

            def ln2_finalize(lg, chunks, mvs2):
                stds = statpool.tile([128, 4], f32, tag="sdb")
                nc.scalar.activation(
                    out=stds[:], in_=mvs2[:, 1::2], func=SQRT,
                    bias=epst[:], scale=1.0,
                )
                nc.vector.reciprocal(out=stds[:], in_=stds[:])
                for i, s2 in enumerate(chunks):
                    ot = outpool.tile([128, D], f32, tag="outp")
                    nc.vector.tensor_scalar(
                        out=ot[:], in0=s2[:],
                        scalar1=mvs2[:, 2 * i:2 * i + 1],
                        scalar2=stds[:, i:i + 1],
                        op0=SUB, op1=MULT,
                    )
                    if ln2_affine:
                        nc.vector.tensor_tensor(out=ot[:], in0=ot[:], in1=g2t[:], op=MULT)
                        nc.vector.tensor_tensor(out=ot[:], in0=ot[:], in1=b2t[:], op=ADD)
                    lr = (lg * 4 + i) * 128
                    out_eng[(lg * 4 + i) % 2].dma_start(out=out[lr:lr + 128, :], in_=ot[:])

            pending = None  # (lg, [(s2, mv) x4]) awaiting finalize
            for lg in range(L // 512):
                hts = []
                for i in range(4):
                    ht = hinpool.tile([128, D], bf16, tag="hin")
                    nc.scalar.dma_start(out=ht[:], in_=hbuf[lg * 4 + i])
                    hts.append(ht)
                # hc[:, d, i*128:(i+1)*128] = h_i[:, d*128:(d+1)*128].T
                hct = hcpool.tile([128, CD, 512], bf16, tag="hcp")
                for dd in range(CD):
                    pt = psp.tile([128, 512], bf16, tag="ps", name="psT")
                    for i in range(4):
                        nc.tensor.transpose(
                            pt[:, i * 128:(i + 1) * 128],
                            hts[i][:, dd * 128:(dd + 1) * 128],
                            ident[:],
                        )
                    nc.vector.tensor_copy(out=hct[:, dd, :], in_=pt[:])
                # MLP1 + gelu (bias fused on ScalarE); hids hidden-major
                hidt = hidpool.tile([128, JD, 512], bf16, tag="hid")
                for j in range(JD):
                    ps = psp.tile([128, 512], f32, tag="ps", name="psA")
                    for dd in range(CD):
                        nc.tensor.matmul(
                            ps[:],
                            w1t[:, dd, j * 128:(j + 1) * 128],
                            hct[:, dd, :],
                            start=(dd == 0),
                            stop=(dd == CD - 1),
                        )
                    nc.scalar.activation(
                        out=hidt[:, j, :], in_=ps[:], func=GELU,
                        bias=b1ct[:, j:j + 1], scale=1.0,
                    )
                if pending is not None:
                    ln2_finalize(*pending)
                # MLP2 directly in L-major: stationary = hids slice, moving = w2
                chunks = []
                mvs2 = statpool.tile([128, 8], f32, tag="mvb")
                for i in range(4):
                    ps = psp.tile([128, D], f32, tag="ps", name="psB")
                    for j in range(JD):
                        nc.tensor.matmul(
                            ps[:],
                            hidt[:, j, i * 128:(i + 1) * 128],
                            w2t[:, j, :],
                            start=(j == 0),
                            stop=(j == JD - 1),
                        )
                    s2 = s2pool.tile([128, D], bf16, tag="s2p")
                    nc.vector.tensor_tensor(
                        out=s2[:], in0=ps[:], in1=hts[i][:], op=ADD,
                    )
                    if b2_nonzero:
                        nc.vector.tensor_tensor(
                            out=s2[:], in0=s2[:], in1=b2bt[:], op=ADD,
                        )
                    stats = statpool.tile([128, 6], f32, tag="st6b")
                    nc.vector.bn_stats(out=stats[:], in_=s2[:])
                    nc.vector.bn_aggr(out=mvs2[:, 2 * i:2 * i + 2], in_=stats[:])
                    chunks.append(s2)
                pending = (lg, chunks, mvs2)
            ln2_finalize(*pending)

    nc.compile()
    return nc


def _prep_inputs(x, chain, W3, b3, W5, b5, W7, b7,
                 mlp_w1, mlp_b1, mlp_w2, mlp_b2,
                 ln1_g, ln1_b, ln2_g, ln2_b):
    import ml_dtypes

    f32 = np.float32
    bf = ml_dtypes.bfloat16
    x = np.asarray(x, f32)
    chain = np.asarray(chain, np.int32)
    flags = (
        not (np.all(np.asarray(ln1_g) == 1.0) and np.all(np.asarray(ln1_b) == 0.0)),
        not (np.all(np.asarray(ln2_g) == 1.0) and np.all(np.asarray(ln2_b) == 0.0)),
        bool(np.any(np.asarray(mlp_b2) != 0.0)),
    )

    # conv weights: per global tap t -> W[:, :, kt].T  (shape [c, o])
    wct = np.empty((15, D, D), f32)
    t = 0
    for W in (W3, W5, W7):
        W = np.asarray(W, f32)
        for k in range(W.shape[2]):
            wct[t] = W[:, :, k].T
            t += 1
    # partition-major flat: wc[p, ((t*CD + c)*D + o)] = W_t[c*128+p, o]
    wc = np.ascontiguousarray(
        wct.reshape(15, CD, 128, D).transpose(2, 0, 1, 3).reshape(128, 15 * CD * D)
    ).astype(bf)

    shared = {
        "wc": wc,
        "cb1": np.ascontiguousarray(np.asarray(b3, f32).reshape(CD, 128).T),
        "cb2": np.ascontiguousarray(np.asarray(b5, f32).reshape(CD, 128).T),
        "w1": np.ascontiguousarray(np.asarray(mlp_w1, f32).reshape(CD, 128, H).transpose(1, 0, 2).reshape(128, CD * H)).astype(bf),
        "b1c": np.ascontiguousarray(np.asarray(mlp_b1, f32).reshape(JD, 128).T),
        "w2": np.ascontiguousarray(np.asarray(mlp_w2, f32).reshape(JD, 128, D).transpose(1, 0, 2).reshape(128, JD * D)).astype(bf),
        "b2m": np.asarray(mlp_b2, f32).reshape(1, D),
    }
    if flags[0]:
        shared["g1r"] = np.asarray(ln1_g, f32).reshape(1, D)
        shared["b1r"] = np.asarray(ln1_b, f32).reshape(1, D)
    if flags[1]:
        shared["g2r"] = np.asarray(ln2_g, f32).reshape(1, D)
        shared["b2r"] = np.asarray(ln2_b, f32).reshape(1, D)

    b7f = np.asarray(b7, f32)
    in_maps = []
    for b in range(B):
        xc = x[b].T  # (D, L)
        xcp = np.zeros((CD, 128, PL), bf)
        xcp[:, :, PAD:PAD + L] = xc.reshape(CD, 128, L).astype(bf)
        xbv = (x[b] + b7f[None, :]).reshape(LCH, 128, D)
        # masks for shifts d in (-3,-2,-1,1,2,3), evaluated at output position
        ce = np.zeros(L + 8, np.int32)
        ce[4:4 + L] = chain[b]
        m = np.empty((6, L), bf)
        for mi, d in enumerate((-3, -2, -1, 1, 2, 3)):
            m[mi] = (ce[4 + d:4 + d + L] == chain[b]).astype(bf)
        im = {"xcp": xcp, "xb": np.ascontiguousarray(xbv),
              "masks": m, **shared}
        in_maps.append(im)
    return in_maps, flags


def kernel(**inputs):
    from concourse.bass_utils import run_bass_kernel_spmd

    in_maps, flags = _prep_inputs(**inputs)
    if flags not in _CACHE:
        _CACHE[flags] = _build_nc(*flags)
    nc = _CACHE[flags]
    res = run_bass_kernel_spmd(nc, in_maps, list(range(NCORES)))
    return np.stack([res.results[b]["out"] for b in range(B)]).astype(np.float32)


# revision 40
# speedup vs baseline: 1.0079x; 1.0079x over previous
"""ConvFormer Trainium2 kernel — data-parallel over B across 8 NeuronCores.

Reference (per batch element b):
    x1 = x[b].T                                   # (D, L) channel-major
    for (W, bias) in ((W3,b3),(W5,b5),(W7,b7)):   # chained masked convs
        x1 = bias + sum_k mask_k * (W[:,:,k] @ shift(x1, k))
    h  = LN(x[b] + x1.T)                          # (L, D)
    out = LN(h + gelu(h @ w1 + b1) @ w2 + b2)

v2 design (per core), all bf16 matmuls (PE streams bf16 at the same
1 col/cycle as f32r, but halves SBUF/DMA/DVE cost). Measured ~681us
(vs 1052us f32r baseline); all matmul phases stream at the 216ns/MM
N=512 hardware floor, 96.5% of bf16 peak FLOPs during the busy window.
  - conv stages ping-pong between two SBUF-resident bf16 tensors
    [128, CD, PL] — no HBM bounce, no inter-conv DMA latency on the
    dependency chain, PE stays warm (HAM never re-throttles).
  - masked conv via masked-input trick: Z_d = shift(x,d) * m_d computed on
    DVE in bf16; all (tap, c) matmuls accumulate in PSUM. One shared 8-bank
    PSUM pool rotates conv blocks (4 banks each, ping-pong) and the MLP
    tiles with zero bank-reuse stalls.
  - convs 1,2 channel-major -> channel-major (copyout fuses the conv bias and
    writes straight into the next stage tensor); conv 3 channel-major ->
    L-major, landing in LayerNorm layout; residual (x + b7, host-folded,
    f32 from HBM) fused into the copyout.
  - LN via bn_stats/bn_aggr; per block/lg the 4 chunks' sqrt+reciprocal are
    batched into one strided op each; h stored bf16, bounced via HBM.
  - MLP: PE-transpose h -> hc; MLP1 out = w1.T @ hc (hidden-major), gelu
    fused on ScalarE; MLP2 uses hids as the *stationary* operand and w2 as
    moving, producing the output directly in L-major — no transpose-back.
  - LN2 finalize (sqrt/recip/normalize/store) deferred one lg so the scalar
    FIFO never blocks the next lg's gelus behind a Sqrt.
  - single flat pool scope: phase 2 instructions interleave with the conv3
    tail; all weights prefetched during phase 1. Startup DMAs split into
    small pieces across the two HWDGE queues (SP=masks, Act=weights+x3) and
    the gpsimd SWDGE queue (x0-2) — first matmul at ~8us after the engine
    preamble, which is the 2-fast-queue bandwidth bound.
  - ScalarE does only Sqrt in phase 1 and Gelu+Sqrt in phase 2 (act-table
    thrash minimized); all PSUM copies/adds on DVE (gpsimd has no PSUM port,
    and its tensor ops are ~5x slower than DVE — keep it to DMA issue only).
"""

import numpy as np

B, L, D = 8, 4096, 512
KS = (3, 5, 7)
EPS = 1e-5
NCORES = 8
PAD = 8            # zero-pad columns each side (PL % 16 == 0, fp8-ready)
PL = PAD + L + PAD
BLK = 512          # conv L-block size
NBLK = L // BLK
CD = D // 128      # 4 channel chunks
LCH = L // 128     # 32 L-chunks of 128
H = 4 * D          # mlp hidden
JD = H // 128      # 16 hidden chunks
GELU_FUNC_NAME = "Gelu_apprx_tanh"  # jax.nn.gelu default is approximate=True

_CACHE = {}


def _build_nc(ln1_affine, ln2_affine, b2_nonzero):
    import concourse.bass as bass
    import concourse.tile as tile
    from concourse import bacc, mybir
    from concourse.masks import make_identity

    f32 = mybir.dt.float32
    bf16 = mybir.dt.bfloat16
    GELU = getattr(mybir.ActivationFunctionType, GELU_FUNC_NAME)
    SQRT = mybir.ActivationFunctionType.Sqrt
    ADD = mybir.AluOpType.add
    SUB = mybir.AluOpType.subtract
    MULT = mybir.AluOpType.mult

    # global tap -> (conv index, shift d); convs use taps [0:3], [3:8], [8:15]
    conv_taps = []
    t0 = 0
    for ki, K in enumerate(KS):
        p = (K - 1) // 2
        conv_taps.append([(t0 + i, i - p) for i in range(K)])
        t0 += K
    NT = t0  # 15
    tap_off = [0, 3, 8]
    # mask row index for shift d (d != 0)
    d2m = {-3: 0, -2: 1, -1: 2, 1: 3, 2: 4, 3: 5}

    nc = bacc.Bacc(None, target_bir_lowering=False)

    xcp = nc.declare_dram_parameter("xcp", [CD, 128, PL], bf16, isOutput=False)
    xb = nc.declare_dram_parameter("xb", [LCH, 128, D], f32, isOutput=False)
    masks = nc.declare_dram_parameter("masks", [6, L], bf16, isOutput=False)
    wc = nc.declare_dram_parameter("wc", [128, NT * CD * D], bf16, isOutput=False)
    cb1 = nc.declare_dram_parameter("cb1", [128, CD], f32, isOutput=False)
    cb2 = nc.declare_dram_parameter("cb2", [128, CD], f32, isOutput=False)
    w1 = nc.declare_dram_parameter("w1", [128, CD * H], bf16, isOutput=False)
    b1c = nc.declare_dram_parameter("b1c", [128, JD], f32, isOutput=False)
    w2 = nc.declare_dram_parameter("w2", [128, JD * D], bf16, isOutput=False)
    b2m = nc.declare_dram_parameter("b2m", [1, D], f32, isOutput=False)
    if ln1_affine:
        g1r = nc.declare_dram_parameter("g1r", [1, D], f32, isOutput=False)
        b1r = nc.declare_dram_parameter("b1r", [1, D], f32, isOutput=False)
    if ln2_affine:
        g2r = nc.declare_dram_parameter("g2r", [1, D], f32, isOutput=False)
        b2r = nc.declare_dram_parameter("b2r", [1, D], f32, isOutput=False)
    out = nc.declare_dram_parameter("out", [L, D], f32, isOutput=True)

    hbuf = nc.dram_tensor("hbuf", [LCH, 128, D], bf16)

    def bcast_row_ap(param, row, col0, n):
        """DMA access pattern: one DRAM row slice broadcast to 128 partitions."""
        src = param[row, col0:col0 + n]
        return bass.AP(tensor=src.tensor, offset=src.offset, ap=[[0, 128]] + list(src.ap))

    from contextlib import ExitStack

    with tile.TileContext(nc) as tc, ExitStack() as es:
        def mkpool(name, bufs, space="SBUF"):
            return es.enter_context(tc.tile_pool(name=name, bufs=bufs, space=space))

        const = mkpool("const", 1)
        stpool = mkpool("stage", 2)
        wpool = mkpool("wconv", 2)
        wmpool = mkpool("wmlp", 1)
        mpool = mkpool("maskp", 4)
        zpool = mkpool("zp", 10)
        xbpool = mkpool("xbp", 2)
        spool = mkpool("sp", 5)
        statpool = mkpool("stats", 8)
        hpool = mkpool("hp", 3)
        hinpool = mkpool("hin", 5)
        hcpool = mkpool("hcp", 1)
        hidpool = mkpool("hid", 1)
        s2pool = mkpool("s2p", 8)
        outpool = mkpool("outp", 3)
        psp = mkpool("psp", 8, space="PSUM")

        if True:
            # ---- input / weight prefetch (many queues, earliest first) ----
            stages = [
                stpool.tile([128, CD, PL], bf16, tag="stage", name=f"stage{i}")
                for i in range(2)
            ]
            sA, sB = stages
            # x lands in stage A as quarter-DMAs, earliest columns first:
            # chunks 0-2 on the gpsimd queue, chunk 3 interleaved with the
            # conv1 weight taps on the scalar queue. Only gpsimd/SP/Act can
            # issue DMAs; sync stays free for the (tiny, critical) masks.
            QL = PL // 4
            def xq(c, q):
                return dict(out=sA[:, c, q * QL:(q + 1) * QL],
                            in_=xcp[c, :, q * QL:(q + 1) * QL])
            # conv1's 3 weight taps load as separate DMAs so the first
            # matmul only waits ~0.5MB; conv3 (7 taps) is split into
            # 4+3-tap tiles so the wconv slot is only 5 taps wide.
            wts = [[], [], []]
            w0 = wpool.tile([128, 3 * CD * D], bf16, tag="wconv", name="wc0")
            wts[0].append((w0, 3))
            TW = CD * D
            nc.scalar.dma_start(out=w0[:, 0:TW], in_=wc[:, 0:TW])
            EL = QL // 2
            for c in range(3):
                nc.gpsimd.dma_start(out=sA[:, c, 0:EL], in_=xcp[c, :, 0:EL])
            for c in range(3):
                nc.gpsimd.dma_start(out=sA[:, c, EL:QL], in_=xcp[c, :, EL:QL])
            for q in range(4):
                if q > 0:
                    for c in range(3):
                        nc.gpsimd.dma_start(**xq(c, q))
                nc.scalar.dma_start(**xq(3, q))
                if q < 2:
                    nc.scalar.dma_start(out=w0[:, (q + 1) * TW:(q + 2) * TW],
                                        in_=wc[:, (q + 1) * TW:(q + 2) * TW])
            for ci, parts, a0 in ((1, (5,), 3 * TW), (2, (4, 3), 8 * TW)):
                for pi, ntap in enumerate(parts):
                    wt = wpool.tile([128, ntap * CD * D], bf16, tag="wconv",
                                    name=f"wc{ci}_{pi}")
                    nc.scalar.dma_start(out=wt[:], in_=wc[:, a0:a0 + ntap * CD * D])
                    a0 += ntap * CD * D
                    wts[ci].append((wt, ntap))
            w1t = wmpool.tile([128, CD, H], bf16, tag="w1")
            for q in range(2):
                nc.scalar.dma_start(
                    out=w1t[:, 2 * q:2 * q + 2, :],
                    in_=w1[:, q * 2 * H:(q + 1) * 2 * H],
                )
            w2t = wmpool.tile([128, JD, D], bf16, tag="w2")
            for q in range(2):
                nc.scalar.dma_start(
                    out=w2t[:, 8 * q:8 * q + 8, :],
                    in_=w2[:, q * 8 * D:(q + 1) * 8 * D],
                )

            # ---- constants ----
            ident = const.tile([128, 128], bf16)
            make_identity(nc, ident)
            epst = const.tile([128, 1], f32)
            nc.vector.memset(epst, EPS)
            cb1t = const.tile([128, CD], f32)
            nc.sync.dma_start(out=cb1t[:], in_=cb1[:])
            cb2t = const.tile([128, CD], f32)
            nc.sync.dma_start(out=cb2t[:], in_=cb2[:])
            b1ct = const.tile([128, JD], f32)
            nc.sync.dma_start(out=b1ct[:], in_=b1c[:])
            if b2_nonzero:
                b2bt = const.tile([128, D], f32)
                nc.sync.dma_start(out=b2bt[:], in_=bcast_row_ap(b2m, 0, 0, D))
            if ln1_affine:
                g1t = const.tile([128, D], f32)
                nc.sync.dma_start(out=g1t[:], in_=bcast_row_ap(g1r, 0, 0, D))
                b1t = const.tile([128, D], f32)
                nc.sync.dma_start(out=b1t[:], in_=bcast_row_ap(b1r, 0, 0, D))
            if ln2_affine:
                g2t = const.tile([128, D], f32)
                nc.sync.dma_start(out=g2t[:], in_=bcast_row_ap(g2r, 0, 0, D))
                b2t = const.tile([128, D], f32)
                nc.sync.dma_start(out=b2t[:], in_=bcast_row_ap(b2r, 0, 0, D))
            # zero the pad columns of both stage tensors once
            for s in stages:
                for c in range(CD):
                    nc.vector.memset(s[:, c, 0:PAD], 0.0)
                    nc.vector.memset(s[:, c, PAD + L:PL], 0.0)

            # ---------------- Phase 1: conv chain + LN1 -> hbuf ----------------
            for ci, (K, taps) in enumerate(zip(KS, conv_taps)):
                src = stages[ci % 2]
                dst = stages[(ci + 1) % 2]  # unused for ci == 2
                tapw = []  # tap -> (weight tile, local tap index)
                for wt, ntap in wts[ci]:
                    for i in range(ntap):
                        tapw.append((wt, i))
                for blk in range(NBLK):
                    l0 = blk * BLK
                    # masked shifted inputs for this block (center tap d=0
                    # reads the stage directly)
                    zcs = {}
                    for ti, (t, d) in enumerate(taps):
                        if d == 0:
                            zcs[ti] = [
                                src[:, c, PAD + l0:PAD + l0 + BLK] for c in range(CD)
                            ]
                        else:
                            mt = mpool.tile([128, BLK], bf16, tag="maskp")
                            nc.sync.dma_start(
                                out=mt[:], in_=bcast_row_ap(masks, d2m[d], l0, BLK)
                            )
                            zs = []
                            for c in range(CD):
                                zt = zpool.tile([128, BLK], bf16, tag="zp")
                                nc.vector.tensor_tensor(
                                    out=zt[:],
                                    in0=src[:, c, PAD + l0 + d:PAD + l0 + d + BLK],
                                    in1=mt[:],
                                    op=MULT,
                                )
                                zs.append(zt)
                            zcs[ti] = zs
                    pss = [
                        psp.tile([128, BLK], f32, tag="ps", name=f"ps{i}")
                        for i in range(4)
                    ]
                    for ti, (t, d) in enumerate(taps):
                        wt, lt = tapw[ti]
                        for c in range(CD):
                            kw = dict(
                                start=(ti == 0 and c == 0),
                                stop=(ti == K - 1 and c == CD - 1),
                                skip_group_check=True,
                            )
                            wbase = (lt * CD + c) * D
                            if ci < 2:
                                # CM: out[o,l]; lhsT = W[c,o]; rhs = Z[c,l]
                                for o in range(CD):
                                    nc.tensor.matmul(
                                        pss[o][:],
                                        wt[:, wbase + o * 128:wbase + (o + 1) * 128],
                                        zcs[ti][c],
                                        **kw,
                                    )
                            else:
                                # LM: out[l,o]; lhsT = Z[c,l-slice]; rhs = W[c,:]
                                for lch in range(BLK // 128):
                                    nc.tensor.matmul(
                                        pss[lch][:],
                                        zcs[ti][c][:, lch * 128:(lch + 1) * 128],
                                        wt[:, wbase:wbase + D],
                                        **kw,
                                    )

                    if ci < 2:
                        bct = cb1t if ci == 0 else cb2t
                        for o in range(CD):
                            nc.vector.tensor_scalar(
                                out=dst[:, o, PAD + l0:PAD + l0 + BLK],
                                in0=pss[o][:],
                                scalar1=bct[:, o:o + 1], scalar2=None, op0=ADD,
                            )
                    else:
                        # LN1 for the 4 chunks of this block: stats first
                        # (DVE), then ONE batched sqrt + reciprocal, then the
                        # 4 normalizes — avoids serializing 4 cross-engine
                        # sqrt round-trips.
                        sts = []
                        mvs = statpool.tile([128, 8], f32, tag="mv")
                        for lch in range(BLK // 128):
                            lg = (l0 + lch * 128) // 128
                            ps = pss[lch]
                            xbt = xbpool.tile([128, D], f32, tag="xbp")
                            nc.gpsimd.dma_start(out=xbt[:], in_=xb[lg])
                            st = spool.tile([128, D], bf16, tag="sp")
                            nc.vector.scalar_tensor_tensor(
                                out=st[:], in0=ps[:], scalar=1.0, in1=xbt[:],
                                op0=MULT, op1=ADD,
                            )
                            sts.append(st)
                            stats = statpool.tile([128, 6], f32, tag="st6")
                            nc.vector.bn_stats(out=stats[:], in_=st[:])
                            nc.vector.bn_aggr(out=mvs[:, 2 * lch:2 * lch + 2], in_=stats[:])
                        stds = statpool.tile([128, 4], f32, tag="sd")
                        nc.scalar.activation(
                            out=stds[:], in_=mvs[:, 1::2], func=SQRT,
                            bias=epst[:], scale=1.0,
                        )
                        nc.vector.reciprocal(out=stds[:], in_=stds[:])
                        for lch in range(BLK // 128):
                            lg = (l0 + lch * 128) // 128
                            ht = hpool.tile([128, D], bf16, tag="hp")
                            nc.vector.tensor_scalar(
                                out=ht[:], in0=sts[lch],
                                scalar1=mvs[:, 2 * lch:2 * lch + 1],
                                scalar2=stds[:, lch:lch + 1],
                                op0=SUB, op1=MULT,
                            )
                            if ln1_affine:
                                nc.vector.tensor_tensor(out=ht[:], in0=ht[:], in1=g1t[:], op=MULT)
                                nc.vector.tensor_tensor(out=ht[:], in0=ht[:], in1=b1t[:], op=ADD)
                            nc.gpsimd.dma_start(out=hbuf[lg], in_=ht[:])

            # ---------------- Phase 2: MLP + LN2 -> out ----------------
            # LN2's finalize (sqrt/recip/normalize/store) for lg is deferred
            # until after lg+1's gelu burst: keeps the scalar FIFO from
            # blocking lg+1's gelus (and hence its MLP2) behind lg's Sqrt.
            out_eng = [nc.scalar, nc.sync]

            def ln2_finalize(lg, chunks, mvs2):
                stds = statpool.tile([128, 4], f32, tag="sdb")
                nc.scalar.activation(
                    out=stds[:], in_=mvs2[:, 1::2], func=SQRT,
                    bias=epst[:], scale=1.0,
                )
                nc.vector.reciprocal(out=stds[:], in_=stds[:])
                for i, s2 in enumerate(chunks):
                    ot = outpool.tile([128, D], f32, tag="outp")
                    nc.vector.tensor_scalar(
                        out=ot[:], in0=s2[:],
                        scalar1=mvs2[:, 2 * i:2 * i + 1],
                        scalar2=stds[:, i:i + 1],
                        op0=SUB, op1=MULT,
                    )
                    if ln2_affine:
                        nc.vector.tensor_tensor(out=ot[:], in0=ot[:], in1=g2t[:], op=MULT)
                        nc.vector.tensor_tensor(out=ot[:], in0=ot[:], in1=b2t[:], op=ADD)
                    lr = (lg * 4 + i) * 128
                    out_eng[(lg * 4 + i) % 2].dma_start(out=out[lr:lr + 128, :], in_=ot[:])

            pending = None  # (lg, [(s2, mv) x4]) awaiting finalize
            for lg in range(L // 512):
                hts = []
                for i in range(4):
                    ht = hinpool.tile([128, D], bf16, tag="hin")
                    nc.scalar.dma_start(out=ht[:], in_=hbuf[lg * 4 + i])
                    hts.append(ht)
                # hc[:, d, i*128:(i+1)*128] = h_i[:, d*128:(d+1)*128].T
                hct = hcpool.tile([128, CD, 512], bf16, tag="hcp")
                for dd in range(CD):
                    pt = psp.tile([128, 512], bf16, tag="ps", name="psT")
                    for i in range(4):
                        nc.tensor.transpose(
                            pt[:, i * 128:(i + 1) * 128],
                            hts[i][:, dd * 128:(dd + 1) * 128],
                            ident[:],
                        )
                    nc.vector.tensor_copy(out=hct[:, dd, :], in_=pt[:])
                # MLP1 + gelu (bias fused on ScalarE); hids hidden-major
                hidt = hidpool.tile([128, JD, 512], bf16, tag="hid")
                for j in range(JD):
                    ps = psp.tile([128, 512], f32, tag="ps", name="psA")
                    for dd in range(CD):
                        nc.tensor.matmul(
                            ps[:],
                            w1t[:, dd, j * 128:(j + 1) * 128],
                            hct[:, dd, :],
                            start=(dd == 0),
                            stop=(dd == CD - 1),
                        )
                    nc.scalar.activation(
                        out=hidt[:, j, :], in_=ps[:], func=GELU,
                        bias=b1ct[:, j:j + 1], scale=1.0,
                    )
                if pending is not None:
                    ln2_finalize(*pending)
                # MLP2 directly in L-major: stationary = hids slice, moving = w2
                last_lg = lg == L // 512 - 1
                chunks = []
                mvs2 = statpool.tile([128, 8], f32, tag="mvb")
                for i in range(4):
                    ps = psp.tile([128, D], f32, tag="ps", name="psB")
                    for j in range(JD):
                        nc.tensor.matmul(
                            ps[:],
                            hidt[:, j, i * 128:(i + 1) * 128],
                            w2t[:, j, :],
                            start=(j == 0),
                            stop=(j == JD - 1),
                        )
                    s2 = s2pool.tile([128, D], bf16, tag="s2p")
                    nc.vector.tensor_tensor(
                        out=s2[:], in0=ps[:], in1=hts[i][:], op=ADD,
                    )
                    if b2_nonzero:
                        nc.vector.tensor_tensor(
                            out=s2[:], in0=s2[:], in1=b2bt[:], op=ADD,
                        )
                    stats = statpool.tile([128, 6], f32, tag="st6b")
                    nc.vector.bn_stats(out=stats[:], in_=s2[:])
                    nc.vector.bn_aggr(out=mvs2[:, 2 * i:2 * i + 2], in_=stats[:])
                    if last_lg:
                        # per-chunk finalize for the last lg only: chunks 0-2
                        # normalize + store while the remaining MLP2 matmuls
                        # stream; only chunk 3's short chain trails the end.
                        std1 = statpool.tile([128, 1], f32, tag="sdb", name="std1")
                        nc.scalar.activation(
                            out=std1[:], in_=mvs2[:, 2 * i + 1:2 * i + 2],
                            func=SQRT, bias=epst[:], scale=1.0,
                        )
                        nc.vector.reciprocal(out=std1[:], in_=std1[:])
                        ot = outpool.tile([128, D], f32, tag="outp")
                        nc.vector.tensor_scalar(
                            out=ot[:], in0=s2[:],
                            scalar1=mvs2[:, 2 * i:2 * i + 1],
                            scalar2=std1[:],
                            op0=SUB, op1=MULT,
                        )
                        if ln2_affine:
                            nc.vector.tensor_tensor(out=ot[:], in0=ot[:], in1=g2t[:], op=MULT)
                            nc.vector.tensor_tensor(out=ot[:], in0=ot[:], in1=b2t[:], op=ADD)
                        lr = (lg * 4 + i) * 128
                        out_eng[i % 2].dma_start(out=out[lr:lr + 128, :], in_=ot[:])
                    else:
                        chunks.append(s2)
                if not last_lg:
                    pending = (lg, chunks, mvs2)

    nc.compile()
    return nc


def _prep_inputs(x, chain, W3, b3, W5, b5, W7, b7,
                 mlp_w1, mlp_b1, mlp_w2, mlp_b2,
                 ln1_g, ln1_b, ln2_g, ln2_b):
    import ml_dtypes

    f32 = np.float32
    bf = ml_dtypes.bfloat16
    x = np.asarray(x, f32)
    chain = np.asarray(chain, np.int32)
    flags = (
        not (np.all(np.asarray(ln1_g) == 1.0) and np.all(np.asarray(ln1_b) == 0.0)),
        not (np.all(np.asarray(ln2_g) == 1.0) and np.all(np.asarray(ln2_b) == 0.0)),
        bool(np.any(np.asarray(mlp_b2) != 0.0)),
    )

    # conv weights: per global tap t -> W[:, :, kt].T  (shape [c, o])
    wct = np.empty((15, D, D), f32)
    t = 0
    for W in (W3, W5, W7):
        W = np.asarray(W, f32)
        for k in range(W.shape[2]):
            wct[t] = W[:, :, k].T
            t += 1
    # partition-major flat: wc[p, ((t*CD + c)*D + o)] = W_t[c*128+p, o]
    wc = np.ascontiguousarray(
        wct.reshape(15, CD, 128, D).transpose(2, 0, 1, 3).reshape(128, 15 * CD * D)
    ).astype(bf)

    shared = {
        "wc": wc,
        "cb1": np.ascontiguousarray(np.asarray(b3, f32).reshape(CD, 128).T),
        "cb2": np.ascontiguousarray(np.asarray(b5, f32).reshape(CD, 128).T),
        "w1": np.ascontiguousarray(np.asarray(mlp_w1, f32).reshape(CD, 128, H).transpose(1, 0, 2).reshape(128, CD * H)).astype(bf),
        "b1c": np.ascontiguousarray(np.asarray(mlp_b1, f32).reshape(JD, 128).T),
        "w2": np.ascontiguousarray(np.asarray(mlp_w2, f32).reshape(JD, 128, D).transpose(1, 0, 2).reshape(128, JD * D)).astype(bf),
        "b2m": np.asarray(mlp_b2, f32).reshape(1, D),
    }
    if flags[0]:
        shared["g1r"] = np.asarray(ln1_g, f32).reshape(1, D)
        shared["b1r"] = np.asarray(ln1_b, f32).reshape(1, D)
    if flags[1]:
        shared["g2r"] = np.asarray(ln2_g, f32).reshape(1, D)
        shared["b2r"] = np.asarray(ln2_b, f32).reshape(1, D)

    b7f = np.asarray(b7, f32)
    in_maps = []
    for b in range(B):
        xc = x[b].T  # (D, L)
        xcp = np.zeros((CD, 128, PL), bf)
        xcp[:, :, PAD:PAD + L] = xc.reshape(CD, 128, L).astype(bf)
        xbv = (x[b] + b7f[None, :]).reshape(LCH, 128, D)
        # masks for shifts d in (-3,-2,-1,1,2,3), evaluated at output position
        ce = np.zeros(L + 8, np.int32)
        ce[4:4 + L] = chain[b]
        m = np.empty((6, L), bf)
        for mi, d in enumerate((-3, -2, -1, 1, 2, 3)):
            m[mi] = (ce[4 + d:4 + d + L] == chain[b]).astype(bf)
        im = {"xcp": xcp, "xb": np.ascontiguousarray(xbv),
              "masks": m, **shared}
        in_maps.append(im)
    return in_maps, flags


def kernel(**inputs):
    from concourse.bass_utils import run_bass_kernel_spmd

    in_maps, flags = _prep_inputs(**inputs)
    if flags not in _CACHE:
        _CACHE[flags] = _build_nc(*flags)
    nc = _CACHE[flags]
    res = run_bass_kernel_spmd(nc, in_maps, list(range(NCORES)))
    return np.stack([res.results[b]["out"] for b in range(B)]).astype(np.float32)
